# revision 17
# baseline (speedup 1.0000x reference)
"""GCN layer (GCNConv + BatchNorm1d + ReLU + residual) on 8 Trainium2 cores.

v2 — gather-pipeline optimized.

Strategy (dst-sharded, 4-queue SWDGE gathers):
  * Nodes sharded by destination across 8 cores (12500 each, padded 12544).
  * Host preprocessing (index-only): sort real edges by (src-quarter, dst
    window, src row); self-loops handled by direct per-window matmuls.
  * Device, per core:
      - preamble per quarter: h = x @ W.T (bf16), scale by dinv[src] on DVE
        in 4-window PSUM-bank groups, store bf16 rows to DRAM stage tables
      - seeds: h_local from xselfT via direct matmuls
      - gathers: dma_gather calls round-robin over all 4 SWDGE queues so the
        four Q7 core pairs generate descriptors concurrently; per-queue
        packed int16 index tables
      - consumers: one-hot S blocks (DVE is_equal, 16 at a time) scatter-
        matmul gathered rows into per-window PSUM, evict-add into SBUF agg
      - after the last chunk's call per window: scale by dinv[dst] and
        accumulate BN stats via ones-column matmuls
      - BN stats AllReduce -> affine -> epilogue relu(agg*s+t)+x -> out
"""

import math
from contextlib import ExitStack

import numpy as np

P = 128
G = 16            # one-hot S blocks built per DVE op (interleaved)
CALL_BLOCKS = 32  # max 128-lane blocks per dma_gather call
N_QUEUES = 4
BN_EPS = 1e-5

N_FULL = 100000
N_CORES = 8


# ---------------------------------------------------------------------------
# Host-side index preprocessing (sharding layout only).
# ---------------------------------------------------------------------------
def make_plan(edge_index: np.ndarray, n: int, n_cores: int):
    assert n % n_cores == 0
    n_loc = n // n_cores
    n_win = math.ceil(n_loc / P)
    n_pad = n_win * P

    # windows per quarter (stage tables must have <= 32767 rows for int16 idx)
    base, rem = divmod(n_win, 4)
    qwin = [base + (1 if i < rem else 0) for i in range(4)]
    qstart_w = np.concatenate([[0], np.cumsum(qwin)])  # window index bounds
    rows_q = [qw * P for qw in qwin]
    stage_rows = [n_cores * r for r in rows_q]
    assert all(r <= 32767 for r in stage_rows)

    src = np.asarray(edge_index[0], dtype=np.int64)
    dst = np.asarray(edge_index[1], dtype=np.int64)

    kdst = dst // n_loc
    ld = dst - kdst * n_loc
    w_arr = ld // P
    drel_a = ld % P
    ksrc = src // n_loc
    s_loc = src - ksrc * n_loc
    ws = s_loc // P
    q_arr = np.searchsorted(qstart_w[1:], ws, side="right")
    row_arr = ksrc * np.asarray(rows_q)[q_arr] + (s_loc - qstart_w[q_arr] * P)

    n_groups = 4 * n_win
    gid = q_arr * n_win + w_arr

    # per-(core, q, w) counts -> shared SPMD block structure
    cnt = np.zeros((n_cores, n_groups), dtype=np.int64)
    for k in range(n_cores):
        cnt[k] = np.bincount(gid[kdst == k], minlength=n_groups)
    nblk = -(-cnt.max(axis=0) // P)  # ceil; 0 if all-empty
    assert nblk.max() <= CALL_BLOCKS, "window group exceeds one gather call"
    blk_base = np.concatenate([[0], np.cumsum(nblk)]).astype(np.int64)
    t_blocks = int(blk_base[-1])
    t_blocks_pad = -(-t_blocks // G) * G

    # calls: within each chunk, runs of whole windows with <= CALL_BLOCKS
    # blocks; queue assigned round-robin; idx columns packed per queue
    calls = []  # dict(c, w0, w1, b0, b1, queue, col_off)
    col_off = 0
    ci = 0
    for c in range(4):
        w = 0
        while w < n_win:
            b0 = int(blk_base[c * n_win + w])
            w1 = w
            while w1 < n_win and int(blk_base[c * n_win + w1 + 1]) - b0 <= CALL_BLOCKS:
                w1 += 1
            b1 = int(blk_base[c * n_win + w1])
            if b1 > b0:
                qn = ci % N_QUEUES
                calls.append(dict(c=c, w0=w, w1=w1, b0=b0, b1=b1, queue=qn,
                                  col_off=col_off))
                col_off += (b1 - b0) * P // 16
                ci += 1
            w = max(w1, w + 1)
    q_cols = col_off

    drel_arr = np.full((n_cores, P, t_blocks_pad), -1.0, dtype=np.float32)
    idx_packed = np.zeros((n_cores, P, q_cols), dtype=np.int16)
    indptr_arr = np.zeros((n_cores, n_pad + 1), dtype=np.int32)

    for k in range(n_cores):
        sel = kdst == k
        gk = gid[sel]
        rk = row_arr[sel]
        dk = drel_a[sel]
        order = np.lexsort((rk, gk))
        gk, rk, dk = gk[order], rk[order], dk[order]

        # lane position: block-structure offset + position within group
        gstart = np.searchsorted(gk, np.arange(n_groups))
        j = np.arange(len(gk))
        pos_in_g = j - gstart[gk]
        lane = blk_base[gk] * P + pos_in_g

        lanes_flat = np.zeros(t_blocks * P, dtype=np.int16)
        lanes_flat[lane] = rk.astype(np.int16)
        # pad lanes within each group repeat row 0 (valid row; drel -1)
        drel_flat = np.full(t_blocks * P, -1.0, dtype=np.float32)
        drel_flat[lane] = dk.astype(np.float32)

        drel_arr[k, :, :t_blocks] = drel_flat.reshape(t_blocks, P).T

        # pack idx lanes per call into the call's queue partition band:
        # [16, L/16] into partitions [32q, 32q+16) and [32q+16, 32q+32)
        for cl in calls:
            L = (cl["b1"] - cl["b0"]) * P
            seg = lanes_flat[cl["b0"] * P: cl["b1"] * P]
            w16 = seg.reshape(L // 16, 16).T  # [16, L/16]
            for rep in range(8):
                p0 = 16 * rep
                idx_packed[k, p0:p0 + 16,
                           cl["col_off"]: cl["col_off"] + L // 16] = w16

        counts = np.bincount(ld[sel], minlength=n_pad)
        counts[:n_loc] += 1  # self-loops
        indptr_arr[k, 1:] = np.cumsum(counts).astype(np.int32)

    return dict(
        n=n, n_cores=n_cores, n_loc=n_loc, n_win=n_win, n_pad=n_pad,
        qwin=qwin, qstart_w=qstart_w, rows_q=rows_q, stage_rows=stage_rows,
        nblk=nblk.reshape(4, n_win), blk_base=blk_base, t_blocks=t_blocks,
        t_blocks_pad=t_blocks_pad, calls=calls, q_cols=q_cols,
        drel_arr=drel_arr, idx_packed=idx_packed, indptr_arr=indptr_arr,
    )


# ---------------------------------------------------------------------------
# Device program
# ---------------------------------------------------------------------------
def build_nc(plan):
    import concourse.bacc as bacc
    import concourse.mybir as mybir
    import concourse.tile as tile
    from concourse import library_config

    f32 = mybir.dt.float32
    bf16 = mybir.dt.bfloat16
    i32 = mybir.dt.int32
    i16 = mybir.dt.int16
    AF = mybir.ActivationFunctionType
    OP = mybir.AluOpType

    n = plan["n"]
    n_cores = plan["n_cores"]
    n_win = plan["n_win"]
    n_pad = plan["n_pad"]
    qwin = plan["qwin"]
    qstart_w = plan["qstart_w"]
    stage_rows = plan["stage_rows"]
    blk_base = plan["blk_base"]
    t_blocks_pad = plan["t_blocks_pad"]
    calls = plan["calls"]
    q_cols = plan["q_cols"]

    nc = bacc.Bacc("TRN2", target_bir_lowering=False, debug=False,
                   num_devices=n_cores, num_swdge_queues=N_QUEUES)

    xT_d = nc.dram_tensor("xT", [P, n_cores * n_pad], bf16, kind="ExternalInput")
    xselfT_d = nc.dram_tensor("xselfT", [P, n_pad], bf16, kind="ExternalInput")
    xloc_d = nc.dram_tensor("xloc", [P, n_win * P], bf16, kind="ExternalInput")
    wt = nc.dram_tensor("wt", [P, P], bf16, kind="ExternalInput")
    deg_l_d = nc.dram_tensor("deg_l", [P, n_win], i32, kind="ExternalInput")
    deg_f_d = nc.dram_tensor("deg_f", [P, n_cores * n_win], i32,
                             kind="ExternalInput")
    idxs_d = nc.dram_tensor("idxs", [P, q_cols], i16, kind="ExternalInput")
    drel_d = nc.dram_tensor("drel", [P, t_blocks_pad], bf16, kind="ExternalInput")
    iota_d = nc.dram_tensor("iota", [P, G * P], bf16, kind="ExternalInput")
    ident_d = nc.dram_tensor("ident", [P, P], f32, kind="ExternalInput")
    gam = nc.dram_tensor("gam", [P, 1], f32, kind="ExternalInput")
    bet = nc.dram_tensor("bet", [P, 1], f32, kind="ExternalInput")
    out_d = nc.dram_tensor("out", [P, n_win * P], f32, kind="ExternalOutput")

    rg = [list(range(n_cores))]

    with tile.TileContext(nc) as tc, ExitStack() as ctx:
        nc.gpsimd.load_library(library_config.mlp)

        const = ctx.enter_context(tc.tile_pool(name="const", bufs=1))
        work = ctx.enter_context(tc.tile_pool(name="work", bufs=2))
        ep = ctx.enter_context(tc.tile_pool(name="ep", bufs=2))
        gtp = ctx.enter_context(tc.tile_pool(name="gtp", bufs=5))
        sp = ctx.enter_context(tc.tile_pool(name="sp", bufs=4))
        pre_ps = ctx.enter_context(tc.tile_pool(name="pre_ps", bufs=2, space="PSUM"))
        win_ps = ctx.enter_context(tc.tile_pool(name="win_ps", bufs=4, space="PSUM"))
        st_ps = ctx.enter_context(tc.tile_pool(name="st_ps", bufs=1, space="PSUM"))
        dram = ctx.enter_context(tc.tile_pool(name="dram", bufs=1, space="DRAM"))

        # ---- constants / inputs resident in SBUF
        wt_sb = const.tile([P, P], bf16)
        nc.sync.dma_start(out=wt_sb[:], in_=wt[:, :])
        iota_sb = const.tile([P, G * P], bf16)
        nc.sync.dma_start(out=iota_sb[:], in_=iota_d[:, :])
        ident_sb = const.tile([P, P], f32)
        nc.sync.dma_start(out=ident_sb[:], in_=ident_d[:, :])
        gam_sb = const.tile([P, 1], f32)
        nc.sync.dma_start(out=gam_sb[:], in_=gam[:, :])
        bet_sb = const.tile([P, 1], f32)
        nc.sync.dma_start(out=bet_sb[:], in_=bet[:, :])
        drel_sb = const.tile([P, t_blocks_pad], bf16)
        nc.sync.dma_start(out=drel_sb[:], in_=drel_d[:, :])
        idx_sb = const.tile([P, q_cols], i16)
        nc.sync.dma_start(out=idx_sb[:], in_=idxs_d[:, :])

        # ---- degree -> dinv  [node%128, window]
        n_gwin = n_cores * n_win
        deg_i = const.tile([P, n_win], i32)
        nc.sync.dma_start(out=deg_i[:], in_=deg_l_d[:, :])
        dinv_c = const.tile([P, n_win], f32)
        nc.vector.tensor_copy(dinv_c[:], deg_i[:])
        nc.vector.tensor_scalar_max(dinv_c[:], dinv_c[:], 1.0)
        nc.scalar.sqrt(dinv_c[:], dinv_c[:])
        nc.vector.reciprocal(dinv_c[:], dinv_c[:])

        degf = const.tile([P, n_gwin], i32)
        nc.sync.dma_start(out=degf[:], in_=deg_f_d[:, :])
        dinv_f = const.tile([P, n_gwin], f32)
        nc.vector.tensor_copy(dinv_f[:], degf[:])
        nc.vector.tensor_scalar_max(dinv_f[:], dinv_f[:], 1.0)
        nc.scalar.sqrt(dinv_f[:], dinv_f[:])
        nc.vector.reciprocal(dinv_f[:], dinv_f[:])

        hs_q = [dram.tile([stage_rows[q], P], bf16, name=f"hs_q{q}")
                for q in range(4)]
        agg = const.tile([P, n_win * P], f32)
        ones_col = const.tile([P, 1], f32)
        nc.vector.memset(ones_col[:], 1.0)
        sum_ps = st_ps.tile([P, 1], f32, tag="sum")
        sq_ps = st_ps.tile([P, 1], f32, tag="sq")

        s_cache = {}

        def s_build(bb):
            # interleaved layout: s[p, rep*G + c] = (drel[p, bb*G+c] == rep)
            if bb in s_cache:
                return
            s8 = sp.tile([P, G * P], bf16, tag="s8")
            in0 = (drel_sb[:, bb * G:(bb + 1) * G]
                   .rearrange("p (o g) -> p o g", o=1)
                   .to_broadcast([P, P, G]))
            nc.vector.tensor_tensor(
                out=s8[:].rearrange("p (r g) -> p r g", g=G),
                in0=in0, in1=iota_sb[:].rearrange("p (r g) -> p r g", g=G),
                op=OP.is_equal)
            for old in [k for k in s_cache if k <= bb - 4]:
                del s_cache[old]
            s_cache[bb] = s8

        def s_tile(b):
            bb = b // G
            s_build(bb)
            return s_cache[bb][:].rearrange("p (r g) -> p g r", g=G)[:, b % G, :]

        def emit_build_k(c, k):
            nw = qwin[c]
            g0 = k * n_win + int(qstart_w[c])
            xt_pc = work.tile([P, nw * P], bf16, tag="xt_pc")
            nc.sync.dma_start(out=xt_pc[:],
                              in_=xT_d[:, g0 * P:(g0 + nw) * P])
            stg = work.tile([P, nw * P], bf16, tag="stg")
            for w4 in range(0, nw, 4):
                m = min(4, nw - w4)
                php = pre_ps.tile([P, 4 * P], f32, tag="php")
                for j in range(m):
                    nc.tensor.matmul(
                        out=php[:, j * P:(j + 1) * P],
                        lhsT=xt_pc[:, (w4 + j) * P:(w4 + j + 1) * P],
                        rhs=wt_sb[:], start=True, stop=True)
                dv = (dinv_f[:, g0 + w4:g0 + w4 + m]
                      .rearrange("p (f o) -> p f o", o=1)
                      .to_broadcast([P, m, P]))
                nc.vector.tensor_tensor(
                    out=stg[:, w4 * P:(w4 + m) * P]
                        .rearrange("p (f v) -> p f v", v=P),
                    in0=php[:, :m * P].rearrange("p (f v) -> p f v", v=P),
                    in1=dv, op=OP.mult)
            r0 = k * nw * P
            nc.scalar.dma_start(
                out=hs_q[c][r0:r0 + nw * P, :].rearrange(
                    "(wi p) f -> p wi f", p=P),
                in_=stg[:].rearrange("p (wi f) -> p wi f", f=P))

        def emit_build(c):
            for k in range(n_cores):
                emit_build_k(c, k)

        def emit_seeds():
            # self-loop seed agg_w = h_local_w * dinv (final dinv[dst] scale
            # makes the self term h*dinv^2)
            for q4 in range(4):
                nw = qwin[q4]
                r0 = int(qstart_w[q4]) * P
                xs = work.tile([P, nw * P], bf16, tag="xt_pc")
                nc.sync.dma_start(out=xs[:], in_=xselfT_d[:, r0:r0 + nw * P])
                for wi in range(nw):
                    w = int(qstart_w[q4]) + wi
                    ph = pre_ps.tile([P, 4 * P], f32, tag="php")
                    nc.tensor.matmul(out=ph[:, 0:P],
                                     lhsT=xs[:, wi * P:(wi + 1) * P],
                                     rhs=wt_sb[:], start=True, stop=True)
                    nc.scalar.activation(out=agg[:, w * P:(w + 1) * P],
                                         in_=ph[:, 0:P], func=AF.Copy,
                                         scale=dinv_c[:, w:w + 1])

        def emit_final(w0, w1):
            # scale by dinv[dst] (4-window DVE groups) + BN stat matmuls
            for wg in range(w0, w1, 4):
                m = min(4, w1 - wg)
                a4 = agg[:, wg * P:(wg + m) * P]
                dv = (dinv_c[:, wg:wg + m]
                      .rearrange("p (f o) -> p f o", o=1)
                      .to_broadcast([P, m, P]))
                nc.vector.tensor_tensor(
                    out=a4.rearrange("p (f v) -> p f v", v=P),
                    in0=a4.rearrange("p (f v) -> p f v", v=P),
                    in1=dv, op=OP.mult)
                sqt = work.tile([P, 4 * P], f32, tag="sqt")
                nc.scalar.activation(out=sqt[:, :m * P], in_=a4, func=AF.Square)
                for j in range(m):
                    w = wg + j
                    nc.tensor.matmul(out=sum_ps[:],
                                     lhsT=agg[:, w * P:(w + 1) * P],
                                     rhs=ones_col[:],
                                     start=(w == 0), stop=(w == n_win - 1))
                    nc.tensor.matmul(out=sq_ps[:],
                                     lhsT=sqt[:, j * P:(j + 1) * P],
                                     rhs=ones_col[:],
                                     start=(w == 0), stop=(w == n_win - 1))

        # ---- main pipeline: all builds + seeds first (keeps the PE stream
        # free of gather-gated consumers until stage tables are done), then
        # the gather pipeline
        for c in range(4):
            emit_build(c)
        emit_seeds()
        final_w = 0  # windows finalized so far (chunk-3 progress)
        for ci, cl in enumerate(calls):
            c = cl["c"]
            w0, w1, b0, b1 = cl["w0"], cl["w1"], cl["b0"], cl["b1"]
            L = (b1 - b0) * P
            gt = gtp.tile([P, CALL_BLOCKS * P], bf16, tag="gt")
            nc.gpsimd.dma_gather(
                gt[:, :L].rearrange("p (b f) -> p b f", f=P),
                hs_q[c][:, :],
                idx_sb[:, cl["col_off"]: cl["col_off"] + L // 16], L, L, P,
                single_packet=False, queue_num=cl["queue"])

            # pre-build S tiles one call ahead so the DVE queue never has a
            # future S-build stuck behind this call's drain-gated adds
            la = calls[min(ci + 1, len(calls) - 1)]
            for bb in range(b0 // G, -(-la["b1"] // G)):
                s_build(bb)

            import os as _os
            if _os.environ.get("NOCONS", "0") == "1":
                nc.vector.tensor_copy(agg[:, 0:1], gt[:, 0:1])
            else:
              for w in range(w0, w1):
                wb0 = int(blk_base[c * n_win + w])
                wb1 = int(blk_base[c * n_win + w + 1])
                if wb1 == wb0:
                    continue
                ps = win_ps.tile([P, P], f32, tag="win")
                for b in range(wb0, wb1):
                    nc.tensor.matmul(
                        out=ps[:], lhsT=s_tile(b),
                        rhs=gt[:, (b - b0) * P:(b - b0 + 1) * P],
                        start=(b == wb0), stop=(b == wb1 - 1))
                a_sl = agg[:, w * P:(w + 1) * P]
                nc.vector.tensor_add(a_sl, ps[:], a_sl)

            # chunk 3: finalize windows as their last contribution lands
            if c == 3:
                emit_final(final_w, w1)
                final_w = w1
        if final_w < n_win:
            emit_final(final_w, n_win)

        # ---- BN statistics all-reduce (stats are [feature, 1] columns)
        stot = const.tile([P, 2], f32)
        nc.scalar.activation(out=stot[:, 0:1], in_=sum_ps[:], func=AF.Copy)
        nc.scalar.activation(out=stot[:, 1:2], in_=sq_ps[:], func=AF.Copy)
        stats_l = dram.tile([P, 2], f32)
        stats_g = dram.tile([P, 2], f32)
        nc.sync.dma_start(out=stats_l[:, :], in_=stot[:])
        nc.gpsimd.collective_compute(
            "AllReduce", mybir.AluOpType.add, replica_groups=rg,
            ins=[stats_l[:].opt()], outs=[stats_g[:].opt()])
        sg = const.tile([P, 2], f32)
        nc.sync.dma_start(out=sg[:], in_=stats_g[:, :])

        # ---- BN affine: s = gamma/std, t = beta - mean*s  ([feature,1] cols)
        mean = const.tile([P, 1], f32)
        nc.vector.tensor_scalar_mul(mean[:], sg[:, 0:1], 1.0 / n)
        var = const.tile([P, 1], f32)
        nc.vector.tensor_scalar_mul(var[:], sg[:, 1:2], 1.0 / n)
        msq = const.tile([P, 1], f32)
        nc.vector.tensor_mul(msq[:], mean[:], mean[:])
        nc.vector.tensor_sub(var[:], var[:], msq[:])
        nc.vector.tensor_scalar_add(var[:], var[:], BN_EPS)
        nc.scalar.sqrt(var[:], var[:])
        s_t = const.tile([P, 1], f32)
        nc.vector.reciprocal(s_t[:], var[:])
        nc.vector.tensor_mul(s_t[:], gam_sb[:], s_t[:])
        t_t = const.tile([P, 1], f32)
        nc.vector.tensor_mul(t_t[:], mean[:], s_t[:])
        nc.vector.tensor_sub(t_t[:], bet_sb[:], t_t[:])

        # transpose s/t columns to rows, then outer-product to [dst, feat],
        # replicated x8 for grouped epilogue
        EG = 8
        onesf = const.tile([1, P], f32)
        nc.vector.memset(onesf[:], 1.0)
        srow8 = const.tile([P, EG * P], f32)
        trow8 = const.tile([P, EG * P], f32)
        for col, row_out in ((s_t, srow8), (t_t, trow8)):
            trp = pre_ps.tile([P, 4 * P], f32, tag="php")
            nc.tensor.matmul(out=trp[0:1, 0:P], lhsT=col[:], rhs=ident_sb[:],
                             start=True, stop=True)
            rrow = work.tile([1, P], f32, tag="rrow")
            nc.scalar.activation(out=rrow[:], in_=trp[0:1, 0:P], func=AF.Copy)
            op_ps = pre_ps.tile([P, 4 * P], f32, tag="php")
            nc.tensor.matmul(out=op_ps[:, 0:P], lhsT=onesf[:], rhs=rrow[:],
                             start=True, stop=True)
            for j in range(EG):
                nc.scalar.activation(out=row_out[:, j * P:(j + 1) * P],
                                     in_=op_ps[:, 0:P], func=AF.Copy)

        # ---- epilogue: out = relu(agg*s + t) + x   (8-window groups)
        for wg in range(0, n_win, EG):
            m = min(EG, n_win - wg)
            xl = ep.tile([P, EG * P], bf16, tag="xl")
            nc.sync.dma_start(out=xl[:, :m * P],
                              in_=xloc_d[:, wg * P:(wg + m) * P])
            a4 = agg[:, wg * P:(wg + m) * P]
            t1 = ep.tile([P, EG * P], f32, tag="t1")
            nc.vector.tensor_mul(t1[:, :m * P], a4, srow8[:, :m * P])
            nc.vector.tensor_add(t1[:, :m * P], t1[:, :m * P], trow8[:, :m * P])
            nc.vector.scalar_tensor_tensor(
                out=t1[:, :m * P], in0=t1[:, :m * P], scalar=0.0,
                in1=xl[:, :m * P], op0=OP.max, op1=OP.add)
            nc.sync.dma_start(out=out_d[:, wg * P:(wg + m) * P],
                              in_=t1[:, :m * P])

    nc.compile()
    return nc


# ---------------------------------------------------------------------------
# Host wrapper
# ---------------------------------------------------------------------------
def _in_maps(plan, x, W, gamma, beta):
    import ml_dtypes

    n_cores = plan["n_cores"]
    n_loc = plan["n_loc"]
    n_pad = plan["n_pad"]
    n_win = plan["n_win"]

    x = np.asarray(x, dtype=np.float32)
    xb = x.astype(ml_dtypes.bfloat16)
    # full transposed x in global padded (rank-major) node order
    xg = np.zeros((n_cores * n_pad, P), dtype=ml_dtypes.bfloat16)
    for k in range(n_cores):
        xg[k * n_pad: k * n_pad + n_loc] = xb[k * n_loc:(k + 1) * n_loc]
    xT_full = np.ascontiguousarray(xg.T)
    # degree tensors in [node%128, window] device layout
    cnts = np.zeros((n_cores, n_pad), dtype=np.int32)
    for k in range(n_cores):
        cnts[k] = np.diff(plan["indptr_arr"][k].astype(np.int64)).astype(np.int32)
    deg_f = np.ascontiguousarray(
        cnts.reshape(n_cores * n_win, P).T.astype(np.int32))
    wt = np.ascontiguousarray(
        np.asarray(W, dtype=np.float32).T.astype(ml_dtypes.bfloat16))
    iota = np.repeat(np.arange(P, dtype=np.float32), G)
    iota = np.tile(iota, (P, 1)).astype(ml_dtypes.bfloat16)
    ident = np.eye(P, dtype=np.float32)
    gam = np.asarray(gamma, dtype=np.float32).reshape(P, 1)
    bet = np.asarray(beta, dtype=np.float32).reshape(P, 1)

    maps = []
    for k in range(n_cores):
        xk = np.zeros((n_pad, P), dtype=ml_dtypes.bfloat16)
        xk[:n_loc] = xb[k * n_loc:(k + 1) * n_loc]
        # SBUF layout [node%128, (window, feat)]
        x_in = np.ascontiguousarray(
            xk.reshape(n_win, P, P).transpose(1, 0, 2).reshape(P, n_win * P))
        xselfT = np.ascontiguousarray(
            xT_full[:, k * n_pad:(k + 1) * n_pad])
        maps.append(dict(
            xT=xT_full, xselfT=xselfT, xloc=x_in, wt=wt,
            deg_l=np.ascontiguousarray(
                cnts[k].reshape(n_win, P).T.astype(np.int32)),
            deg_f=deg_f,
            idxs=np.ascontiguousarray(plan["idx_packed"][k]),
            drel=np.ascontiguousarray(
                plan["drel_arr"][k].astype(ml_dtypes.bfloat16)),
            iota=iota, ident=ident,
            gam=gam, bet=bet,
        ))
    return maps


def run(x, edge_index, W, b, gamma, beta, n=N_FULL, n_cores=N_CORES, trace=False):
    from concourse.bass_utils import run_bass_kernel_spmd

    plan = make_plan(np.asarray(edge_index), n, n_cores)
    nc = build_nc(plan)
    maps = _in_maps(plan, x, W, gamma, beta)
    res = run_bass_kernel_spmd(nc, maps, core_ids=list(range(n_cores)), trace=trace)
    n_loc = plan["n_loc"]
    n_win = plan["n_win"]
    outs = []
    for k in range(n_cores):
        o = res.results[k]["out"]  # [128, n_win*128] node-major [d, f]
        o = o.reshape(P, n_win, P).transpose(1, 0, 2).reshape(n_win * P, P)
        outs.append(o[:n_loc])
    return np.concatenate(outs, axis=0), res


def kernel(x, edge_index, W, b, gamma, beta):
    out, _ = run(x, edge_index, W, b, gamma, beta)
    return out
